# revision 8
# baseline (speedup 1.0000x reference)
"""2-layer GCN encoder as a distributed Bass kernel on 8 TRN2 NeuronCores.

Decomposition (per core, nodes sharded by destination):
  hs1[v] = dinv[v] * (x[v] @ W1)                  (own rows -> split AllGather, bf16)
  S1[d]  = sum_{e: dst=d} hs1[src_e]              (SWDGE dma_gather + one-hot matmul)
  hsr    = dinv * relu(dinv*S1 + b1)              (own rows -> split AllGather, bf16)
  S2[d]  = sum_{e: dst=d} hsr[src_e]
  y[d]   = dinv[d]*(S2[d] @ W2) + b2              (W2 commutes with the sum)

vs the original version:
  - one-hot scatter matrices are generated ON DEVICE (iota + tensor_scalar
    is_equal with a per-partition dst-index scalar) instead of streaming
    ~39MB of precomputed one-hots from HBM per core.
  - self-loops are plain edges in the gather lists (no identity-matmul /
    staging-reload special case).
  - each AllGather is split in two halves (rows [0,1920) and [1920,3750) of
    every core) writing two separate shared tables; edges are partitioned by
    source half so half-A gathers only depend on the first collective and
    overlap the second.
  - x is loaded in column chunks so phase B starts ~4MB earlier.
  - per-core edge lists are sorted by source row for HBM locality, and
    gather segments span dst-block boundaries (uniform 1024-index gathers).
"""

import numpy as np

import concourse.bass as bass
import concourse.bacc as bacc
import concourse.mybir as mybir
import concourse.tile as tile
from concourse import library_config
from concourse.bass_utils import run_bass_kernel_spmd

F32 = mybir.dt.float32
BF16 = mybir.dt.bfloat16
I16 = mybir.dt.int16

NCORES = 8
BLK = 128
N = 30000
NP = N // NCORES          # 3750
NBLK = (NP + BLK - 1) // BLK   # 30
HALF_BLKS = 15
RA = HALF_BLKS * BLK      # 1920 rows in half A
RB = NP - RA              # 1830 rows in half B
# Max 128-index chunks per dma_gather instruction: the SWDGE descriptor
# ring holds only ~100 descriptors per DMA engine and a gather generates
# num_idxs/16 per ring; >=1792 indices hangs the ring-reclaim wait. 1024 is
# known-safe.
MAXCH = 8
NQUEUES = 4
XCHUNK = 1024             # x load column-chunk (8 blocks)


def _cdiv(a, b):
    return (a + b - 1) // b


def preprocess(x, edge_index, ncores=NCORES):
    """Host-side graph partitioning: shard edges by dst core, split by source
    half, sort by source row, build wrapped SWDGE gather indices and the
    per-edge local-dst table used for on-device one-hot generation."""
    import ml_dtypes

    n, IN = x.shape
    assert n == N and N % ncores == 0

    src = np.asarray(edge_index[0], dtype=np.int64)
    dst = np.asarray(edge_index[1], dtype=np.int64)
    # self-loops as ordinary edges (PyG gcn_norm add_self_loops=True)
    loop = np.arange(N, dtype=np.int64)
    src = np.concatenate([src, loop])
    dst = np.concatenate([dst, loop])
    deg = np.bincount(dst, minlength=N).astype(np.float32)

    src_core = src // NP
    src_j = src % NP
    in_a = src_j < RA
    # table rows in the split-AllGather layout
    row_a = src_core * RA + src_j
    row_b = src_core * RB + (src_j - RA)

    per_core_lists = []
    cntA = np.zeros((ncores, NBLK), np.int64)
    cntB = np.zeros((ncores, NBLK), np.int64)
    for i in range(ncores):
        m = (dst >= i * NP) & (dst < (i + 1) * NP)
        es, ed = src[m], dst[m] - i * NP
        ia = in_a[m]
        ra, rb = row_a[m], row_b[m]
        blk = ed // BLK
        dl = ed % BLK
        lists = {}
        for b in range(NBLK):
            mb = blk == b
            for half, rows in (("A", ra), ("B", rb)):
                mh = mb & (ia if half == "A" else ~ia)
                r = rows[mh]
                d = dl[mh]
                o = np.argsort(r, kind="stable")
                lists[(b, half)] = (r[o], d[o])
                if half == "A":
                    cntA[i, b] = r.size
                else:
                    cntB[i, b] = r.size
        per_core_lists.append(lists)

    CHA = np.maximum(1, _cdiv(cntA.max(axis=0), BLK)).astype(np.int64)
    CHB = np.maximum(1, _cdiv(cntB.max(axis=0), BLK)).astype(np.int64)
    cofsA = np.concatenate([[0], np.cumsum(CHA)]).astype(np.int64)
    cofsB = np.concatenate([[0], np.cumsum(CHB)]).astype(np.int64)
    NCHTA = int(CHA.sum())
    NCHTB = int(CHB.sum())
    NCHT = NCHTA + NCHTB
    widths = [min(BLK, NP - b * BLK) for b in range(NBLK)]

    def build_stream(lists, half, CH, ncht):
        rows = np.zeros(ncht * BLK, np.int64)
        dl = np.full(ncht * BLK, -1.0, np.float32)
        c0 = 0
        for b in range(NBLK):
            r, d = lists[(b, half)]
            L = int(CH[b]) * BLK
            rows[c0 * BLK : c0 * BLK + r.size] = r
            dl[c0 * BLK : c0 * BLK + d.size] = d
            c0 += int(CH[b])
        return rows, dl

    per_core = []
    for i in range(ncores):
        ra_s, dla = build_stream(per_core_lists[i], "A", CHA, NCHTA)
        rb_s, dlb = build_stream(per_core_lists[i], "B", CHB, NCHTB)
        rows = np.concatenate([ra_s, rb_s])
        dl = np.concatenate([dla, dlb])
        assert rows.max() < 32768
        # wrapped SWDGE index layout, per gather segment: idx k of a segment
        # sits at [16*rep + k%16, seg_col0 + k//16] for rep in 0..7
        gidx = np.zeros((128, NCHT * 8), np.int16)
        dloc = dl.reshape(NCHT, BLK).T.astype(np.float32)  # [128, NCHT]
        for base, ncht in ((0, NCHTA), (NCHTA, NCHTB)):
            for s0 in range(0, ncht, MAXCH):
                sch = min(MAXCH, ncht - s0)
                L = sch * BLK
                seg = rows[(base + s0) * BLK : (base + s0) * BLK + L]
                wr = seg.reshape(L // 16, 16).T.astype(np.int16)  # [16, L//16]
                gidx[:, (base + s0) * 8 : (base + s0 + sch) * 8] = np.tile(wr, (8, 1))
        degp = np.concatenate(
            [deg[i * NP : (i + 1) * NP], np.ones(NBLK * BLK - NP, np.float32)]
        )
        per_core.append(
            {
                "x_tr": np.ascontiguousarray(x[i * NP : (i + 1) * NP].T),
                "deg_own": np.ascontiguousarray(degp.reshape(NBLK, BLK).T),
                "gidx": gidx,
                "dloc": np.ascontiguousarray(dloc),
            }
        )

    meta = {
        "IN": IN,
        "CHA": [int(c) for c in CHA],
        "CHB": [int(c) for c in CHB],
        "cofsA": [int(c) for c in cofsA],
        "cofsB": [int(c) for c in cofsB],
        "NCHTA": NCHTA,
        "NCHTB": NCHTB,
        "NCHT": NCHT,
        "widths": widths,
    }
    return per_core, meta


def build_nc(meta, HID, OUT, ncores=NCORES):
    IN = meta["IN"]
    widths = meta["widths"]
    NCHT = meta["NCHT"]
    KC = IN // 128
    assert IN % 128 == 0 and HID == 128 and OUT <= 512

    nc = bacc.Bacc(
        "TRN2",
        target_bir_lowering=False,
        debug=False,
        num_devices=ncores,
        num_swdge_queues=NQUEUES,
    )

    x_tr = nc.dram_tensor("x_tr", [IN, NP], F32, kind="ExternalInput")
    w1 = nc.dram_tensor("w1", [IN, HID], F32, kind="ExternalInput")
    b1 = nc.dram_tensor("b1", [1, HID], F32, kind="ExternalInput")
    w2 = nc.dram_tensor("w2", [HID, OUT], F32, kind="ExternalInput")
    b2 = nc.dram_tensor("b2", [1, OUT], F32, kind="ExternalInput")
    deg_own = nc.dram_tensor("deg_own", [128, NBLK], F32, kind="ExternalInput")
    gidx_d = nc.dram_tensor("gidx", [128, NCHT * 8], I16, kind="ExternalInput")
    dloc_d = nc.dram_tensor("dloc", [128, NCHT], F32, kind="ExternalInput")
    iota_d = nc.dram_tensor("iota", [128, 128], BF16, kind="ExternalInput")
    y = nc.dram_tensor("y", [NP, OUT], F32, kind="ExternalOutput")

    hs1_stage = nc.dram_tensor("hs1_stage", [NP, HID], BF16)
    hs1A = nc.dram_tensor("hs1A", [ncores * RA, HID], BF16, addr_space="Shared")
    hs1B = nc.dram_tensor("hs1B", [ncores * RB, HID], BF16, addr_space="Shared")
    hsr_stage = nc.dram_tensor("hsr_stage", [NP, HID], BF16)
    hsrA = nc.dram_tensor("hsrA", [ncores * RA, HID], BF16, addr_space="Shared")
    hsrB = nc.dram_tensor("hsrB", [ncores * RB, HID], BF16, addr_space="Shared")

    rg = [list(range(ncores))]
    qn = [0]

    def next_q():
        q = qn[0]
        qn[0] = (q + 1) % NQUEUES
        return q

    streams = [
        # (base chunk offset, cofs, ncht)
        (0, meta["cofsA"], meta["NCHTA"]),
        (meta["NCHTA"], meta["cofsB"], meta["NCHTB"]),
    ]

    with tile.TileContext(nc) as tc:
        nc.gpsimd.load_library(library_config.mlp)
        with (
            tc.tile_pool(name="const", bufs=1) as constp,
            tc.tile_pool(name="gath", bufs=8) as gathp,
            tc.tile_pool(name="oh", bufs=16) as ohp,
            tc.tile_pool(name="hs", bufs=8) as hsp,
            tc.tile_pool(name="sa", bufs=1) as sap,
            tc.tile_pool(name="ps", bufs=4, space="PSUM") as psp,
            tc.tile_pool(name="pso", bufs=2, space="PSUM") as psop,
        ):
            # ---- constants (DMA program order = priority order) ----
            w1c = []
            for k in range(KC):
                t = constp.tile([128, HID], F32, tag=f"w1c{k}")
                nc.sync.dma_start(out=t[:], in_=w1[k * 128 : (k + 1) * 128, :])
                w1c.append(t)
            # x in column chunks so phase B can start before the full load
            nxt_chunks = _cdiv(NP, XCHUNK)
            xsb = [[None] * nxt_chunks for _ in range(KC)]
            for t_i in range(nxt_chunks):
                c0 = t_i * XCHUNK
                cw = min(XCHUNK, NP - c0)
                for k in range(KC):
                    t = constp.tile([128, cw], F32, tag=f"x{k}_{t_i}")
                    nc.sync.dma_start(
                        out=t[:], in_=x_tr[k * 128 : (k + 1) * 128, c0 : c0 + cw]
                    )
                    xsb[k][t_i] = t
            b1_sb = constp.tile([1, HID], F32, tag="b1")
            nc.sync.dma_start(out=b1_sb[:], in_=b1[:, :])
            dinv_sb = constp.tile([128, NBLK], F32, tag="dinv")
            nc.sync.dma_start(out=dinv_sb[:], in_=deg_own[:, :])
            nc.scalar.sqrt(dinv_sb[:], dinv_sb[:])
            nc.vector.reciprocal(dinv_sb[:], dinv_sb[:])
            gidx_sb = constp.tile([128, NCHT * 8], I16, tag="gidx")
            nc.sync.dma_start(out=gidx_sb[:], in_=gidx_d[:, :])
            dloc_sb = constp.tile([128, NCHT], F32, tag="dloc")
            nc.sync.dma_start(out=dloc_sb[:], in_=dloc_d[:, :])
            w2_sb = constp.tile([HID, OUT], F32, tag="w2")
            nc.sync.dma_start(out=w2_sb[:], in_=w2[:, :])
            b2_sb = constp.tile([1, OUT], F32, tag="b2")
            nc.sync.dma_start(out=b2_sb[:], in_=b2[:, :])

            ones_sb = constp.tile([1, 128], F32, tag="ones")
            nc.vector.memset(ones_sb[:], 1.0)
            pb = psop.tile([128, HID], F32, tag="po")
            nc.tensor.matmul(pb[:], lhsT=ones_sb[:], rhs=b1_sb[:],
                             start=True, stop=True)
            b1_bc = constp.tile([128, HID], F32, tag="b1bc")
            nc.vector.tensor_copy(b1_bc[:], pb[:])
            pb2 = psop.tile([128, OUT], F32, tag="po")
            nc.tensor.matmul(pb2[:], lhsT=ones_sb[:], rhs=b2_sb[:],
                             start=True, stop=True)
            b2_bc = constp.tile([128, OUT], F32, tag="b2bc")
            nc.vector.tensor_copy(b2_bc[:], pb2[:])

            iota_bc = constp.tile([128, 128], BF16, tag="iotb")
            nc.sync.dma_start(out=iota_bc[:], in_=iota_d[:, :])

            # ---- phase B: hs1 = dinv * (x @ W1) for own rows ----
            for b in range(NBLK):
                w = widths[b]
                t_i = (b * BLK) // XCHUNK
                co = b * BLK - t_i * XCHUNK
                ph = psp.tile([128, HID], F32, tag="acc")
                for k in range(KC):
                    nc.tensor.matmul(
                        ph[:w, :],
                        lhsT=xsb[k][t_i][:, co : co + w],
                        rhs=w1c[k][:, :],
                        start=(k == 0),
                        stop=(k == KC - 1),
                    )
                hs1_t = hsp.tile([128, HID], BF16, tag="hs1")
                nc.scalar.activation(
                    hs1_t[:w, :],
                    ph[:w, :],
                    mybir.ActivationFunctionType.Copy,
                    scale=dinv_sb[:w, b : b + 1],
                )
                nc.sync.dma_start(
                    out=hs1_stage[b * BLK : b * BLK + w, :], in_=hs1_t[:w, :]
                )
                if b == HALF_BLKS - 1:
                    nc.gpsimd.collective_compute(
                        "AllGather",
                        mybir.AluOpType.bypass,
                        replica_groups=rg,
                        ins=[hs1_stage[0:RA, :].opt()],
                        outs=[hs1A[:, :].opt()],
                    )
            nc.gpsimd.collective_compute(
                "AllGather",
                mybir.AluOpType.bypass,
                replica_groups=rg,
                ins=[hs1_stage[RA:NP, :].opt()],
                outs=[hs1B[:, :].opt()],
            )

            def gen_oh(ac):
                ohc = ohp.tile([128, 128], BF16, tag="oh")
                nc.vector.tensor_scalar(
                    ohc[:],
                    iota_bc[:],
                    dloc_sb[:, ac : ac + 1],
                    None,
                    op0=mybir.AluOpType.is_equal,
                )
                return ohc

            def do_pass(table, stream_i, feat_major, close_block):
                """One scatter pass over a gather stream.

                feat_major=False: psum[d, f] += oh^T @ g   (layer-1 layout)
                feat_major=True : psum[f, d] += g^T @ oh   (layer-2 layout)
                close_block(b, psum_tile) consumes the finished accumulator.
                """
                base, cofs, ncht = streams[stream_i]
                p = None
                g = None
                for ac in range(ncht):
                    if ac % MAXCH == 0:
                        sch = min(MAXCH, ncht - ac)
                        g = gathp.tile([128, MAXCH, HID], BF16, tag="g")
                        nc.gpsimd.dma_gather(
                            g[:, :sch, :],
                            table.ap(),
                            gidx_sb[:, (base + ac) * 8 : (base + ac + sch) * 8],
                            sch * 128,
                            sch * 128,
                            HID,
                            queue_num=next_q(),
                        )
                    b = int(np.searchsorted(cofs, ac, side="right")) - 1
                    w = widths[b]
                    first = ac == cofs[b]
                    last = ac == cofs[b + 1] - 1
                    if first:
                        p = psp.tile([128, 128], F32, tag="acc")
                    ohc = gen_oh(base + ac)
                    if feat_major:
                        nc.tensor.matmul(
                            p[:, :w],
                            lhsT=g[:, ac % MAXCH, :],
                            rhs=ohc[:, :w],
                            start=first,
                            stop=last,
                        )
                    else:
                        nc.tensor.matmul(
                            p[:w, :],
                            lhsT=ohc[:, :w],
                            rhs=g[:, ac % MAXCH, :],
                            start=first,
                            stop=last,
                        )
                    if last:
                        close_block(b, p)

            # ---- phase D: layer-1 aggregation -> hsr ----
            sa1 = [sap.tile([128, HID], F32, tag=f"sa1_{b}", name=f"sa1_{b}") for b in range(NBLK)]

            def d_close_a(b, p):
                w = widths[b]
                nc.scalar.activation(
                    sa1[b][:w, :], p[:w, :], mybir.ActivationFunctionType.Copy
                )

            def d_close_b(b, p):
                w = widths[b]
                t0 = hsp.tile([128, HID], F32, tag="t0")
                nc.vector.tensor_tensor(
                    out=t0[:w, :], in0=p[:w, :], in1=sa1[b][:w, :],
                    op=mybir.AluOpType.add,
                )
                t1 = hsp.tile([128, HID], F32, tag="t1")
                nc.scalar.activation(
                    t1[:w, :], t0[:w, :],
                    mybir.ActivationFunctionType.Copy,
                    scale=dinv_sb[:w, b : b + 1],
                )
                t2 = hsp.tile([128, HID], F32, tag="t2")
                nc.vector.tensor_tensor(
                    out=t2[:w, :], in0=t1[:w, :], in1=b1_bc[:w, :],
                    op=mybir.AluOpType.add,
                )
                hsr_t = hsp.tile([128, HID], BF16, tag="hsr")
                nc.scalar.activation(
                    hsr_t[:w, :], t2[:w, :],
                    mybir.ActivationFunctionType.Relu,
                    scale=dinv_sb[:w, b : b + 1],
                )
                nc.sync.dma_start(
                    out=hsr_stage[b * BLK : b * BLK + w, :], in_=hsr_t[:w, :]
                )
                if b == HALF_BLKS - 1:
                    nc.gpsimd.collective_compute(
                        "AllGather",
                        mybir.AluOpType.bypass,
                        replica_groups=rg,
                        ins=[hsr_stage[0:RA, :].opt()],
                        outs=[hsrA[:, :].opt()],
                    )
                if b == NBLK - 1:
                    nc.gpsimd.collective_compute(
                        "AllGather",
                        mybir.AluOpType.bypass,
                        replica_groups=rg,
                        ins=[hsr_stage[RA:NP, :].opt()],
                        outs=[hsrB[:, :].opt()],
                    )

            do_pass(hs1A, 0, False, d_close_a)
            do_pass(hs1B, 1, False, d_close_b)

            # ---- phase F: layer-2 aggregation -> y ----
            sa2 = [sap.tile([128, 128], F32, tag=f"sa2_{b}", name=f"sa2_{b}") for b in range(NBLK)]

            def f_close_a(b, p):
                w = widths[b]
                nc.scalar.activation(
                    sa2[b][:, :w], p[:, :w], mybir.ActivationFunctionType.Copy
                )

            def f_close_b(b, p):
                w = widths[b]
                aggT = hsp.tile([128, 128], F32, tag="aggT")
                nc.vector.tensor_tensor(
                    out=aggT[:, :w], in0=p[:, :w], in1=sa2[b][:, :w],
                    op=mybir.AluOpType.add,
                )
                po = psop.tile([128, OUT], F32, tag="po")
                nc.tensor.matmul(
                    po[:w, :], lhsT=aggT[:, :w], rhs=w2_sb[:, :],
                    start=True, stop=True,
                )
                o1 = hsp.tile([128, OUT], F32, tag="o1")
                nc.scalar.activation(
                    o1[:w, :], po[:w, :],
                    mybir.ActivationFunctionType.Copy,
                    scale=dinv_sb[:w, b : b + 1],
                )
                yt = hsp.tile([128, OUT], F32, tag="yt")
                nc.vector.tensor_tensor(
                    out=yt[:w, :], in0=o1[:w, :], in1=b2_bc[:w, :],
                    op=mybir.AluOpType.add,
                )
                nc.sync.dma_start(out=y[b * BLK : b * BLK + w, :], in_=yt[:w, :])

            do_pass(hsrA, 0, True, f_close_a)
            do_pass(hsrB, 1, True, f_close_b)

    nc.compile()
    return nc


def make_in_maps(per_core, W1, b1, W2, b2):
    import ml_dtypes

    W1 = np.ascontiguousarray(np.asarray(W1, np.float32))
    W2 = np.ascontiguousarray(np.asarray(W2, np.float32))
    b1 = np.asarray(b1, np.float32).reshape(1, -1)
    b2 = np.asarray(b2, np.float32).reshape(1, -1)
    iota = np.broadcast_to(
        np.arange(128, dtype=np.float32), (128, 128)
    ).astype(ml_dtypes.bfloat16)
    return [
        {
            "x_tr": pc["x_tr"],
            "w1": W1,
            "b1": b1,
            "w2": W2,
            "b2": b2,
            "deg_own": pc["deg_own"],
            "gidx": pc["gidx"],
            "dloc": pc["dloc"],
            "iota": np.ascontiguousarray(iota),
        }
        for pc in per_core
    ]


def kernel_run(x, edge_index, W1, b1, W2, b2, trace=False, tmpdir=None):
    x = np.ascontiguousarray(np.asarray(x, np.float32))
    per_core, meta = preprocess(x, edge_index)
    HID = np.asarray(W1).shape[1]
    OUT = np.asarray(W2).shape[1]
    nc = build_nc(meta, HID, OUT)
    in_maps = make_in_maps(per_core, W1, b1, W2, b2)
    res = run_bass_kernel_spmd(
        nc, in_maps, core_ids=list(range(NCORES)), trace=trace, tmpdir=tmpdir
    )
    out = np.concatenate([r["y"] for r in res.results], axis=0)
    return out, res


def kernel(x, edge_index, W1, b1, W2, b2):
    out, _ = kernel_run(x, edge_index, W1, b1, W2, b2)
    return out


# revision 9
# speedup vs baseline: 1.3533x; 1.3533x over previous
"""2-layer GCN encoder as a distributed Bass kernel on 8 TRN2 NeuronCores.

Decomposition (per core, nodes sharded by destination):
  hs1[v] = dinv[v] * (x[v] @ W1)                  (own rows -> split AllGather, bf16)
  S1[d]  = sum_{e: dst=d} hs1[src_e]              (SWDGE dma_gather + one-hot matmul)
  hsr    = dinv * relu(dinv*S1 + b1)              (own rows -> split AllGather, bf16)
  S2[d]  = sum_{e: dst=d} hsr[src_e]
  y[d]   = dinv[d]*(S2[d] @ W2) + b2              (W2 commutes with the sum)

vs the original version:
  - one-hot scatter matrices are generated ON DEVICE (iota + tensor_scalar
    is_equal with a per-partition dst-index scalar) instead of streaming
    ~39MB of precomputed one-hots from HBM per core.
  - self-loops are plain edges in the gather lists (no identity-matmul /
    staging-reload special case).
  - each AllGather is split in two halves (rows [0,1920) and [1920,3750) of
    every core) writing two separate shared tables; edges are partitioned by
    source half so half-A gathers only depend on the first collective and
    overlap the second.
  - x is loaded in column chunks so phase B starts ~4MB earlier.
  - per-core edge lists are sorted by source row for HBM locality, and
    gather segments span dst-block boundaries (uniform 1024-index gathers).
"""

import numpy as np

import concourse.bass as bass
import concourse.bacc as bacc
import concourse.mybir as mybir
import concourse.tile as tile
from concourse import library_config
from concourse.bass_utils import run_bass_kernel_spmd

F32 = mybir.dt.float32
BF16 = mybir.dt.bfloat16
I16 = mybir.dt.int16

NCORES = 8
BLK = 128
N = 30000
NP = N // NCORES          # 3750
NBLK = (NP + BLK - 1) // BLK   # 30
HALF_BLKS = 15
RA = HALF_BLKS * BLK      # 1920 rows in half A
RB = NP - RA              # 1830 rows in half B
# Max 128-index chunks per dma_gather instruction: the SWDGE descriptor
# ring holds only ~100 descriptors per DMA engine and a gather generates
# num_idxs/16 per ring; >=1792 indices hangs the ring-reclaim wait. 1024 is
# known-safe.
MAXCH = 8
NQUEUES = 4
XCHUNK = 1024             # x load column-chunk (8 blocks)


def _cdiv(a, b):
    return (a + b - 1) // b


def preprocess(x, edge_index, ncores=NCORES):
    """Host-side graph partitioning: shard edges by dst core, split by source
    half, sort by source row, build wrapped SWDGE gather indices and the
    per-edge local-dst table used for on-device one-hot generation."""
    import ml_dtypes

    n, IN = x.shape
    assert n == N and N % ncores == 0

    src = np.asarray(edge_index[0], dtype=np.int64)
    dst = np.asarray(edge_index[1], dtype=np.int64)
    # self-loops as ordinary edges (PyG gcn_norm add_self_loops=True)
    loop = np.arange(N, dtype=np.int64)
    src = np.concatenate([src, loop])
    dst = np.concatenate([dst, loop])
    deg = np.bincount(dst, minlength=N).astype(np.float32)

    src_core = src // NP
    src_j = src % NP
    in_a = src_j < RA
    # table rows in the split-AllGather layout
    row_a = src_core * RA + src_j
    row_b = src_core * RB + (src_j - RA)

    per_core_lists = []
    cntA = np.zeros((ncores, NBLK), np.int64)
    cntB = np.zeros((ncores, NBLK), np.int64)
    for i in range(ncores):
        m = (dst >= i * NP) & (dst < (i + 1) * NP)
        es, ed = src[m], dst[m] - i * NP
        ia = in_a[m]
        ra, rb = row_a[m], row_b[m]
        blk = ed // BLK
        dl = ed % BLK
        lists = {}
        for b in range(NBLK):
            mb = blk == b
            for half, rows in (("A", ra), ("B", rb)):
                mh = mb & (ia if half == "A" else ~ia)
                r = rows[mh]
                d = dl[mh]
                o = np.argsort(r, kind="stable")
                lists[(b, half)] = (r[o], d[o])
                if half == "A":
                    cntA[i, b] = r.size
                else:
                    cntB[i, b] = r.size
        per_core_lists.append(lists)

    CHA = np.maximum(1, _cdiv(cntA.max(axis=0), BLK)).astype(np.int64)
    CHB = np.maximum(1, _cdiv(cntB.max(axis=0), BLK)).astype(np.int64)
    cofsA = np.concatenate([[0], np.cumsum(CHA)]).astype(np.int64)
    cofsB = np.concatenate([[0], np.cumsum(CHB)]).astype(np.int64)
    NCHTA = int(CHA.sum())
    NCHTB = int(CHB.sum())
    NCHT = NCHTA + NCHTB
    widths = [min(BLK, NP - b * BLK) for b in range(NBLK)]

    def build_stream(lists, half, CH, ncht):
        rows = np.zeros(ncht * BLK, np.int64)
        dl = np.full(ncht * BLK, -1.0, np.float32)
        c0 = 0
        for b in range(NBLK):
            r, d = lists[(b, half)]
            L = int(CH[b]) * BLK
            rows[c0 * BLK : c0 * BLK + r.size] = r
            dl[c0 * BLK : c0 * BLK + d.size] = d
            c0 += int(CH[b])
        return rows, dl

    per_core = []
    for i in range(ncores):
        ra_s, dla = build_stream(per_core_lists[i], "A", CHA, NCHTA)
        rb_s, dlb = build_stream(per_core_lists[i], "B", CHB, NCHTB)
        rows = np.concatenate([ra_s, rb_s])
        dl = np.concatenate([dla, dlb])
        assert rows.max() < 32768
        # wrapped SWDGE index layout, per gather segment: idx k of a segment
        # sits at [16*rep + k%16, seg_col0 + k//16] for rep in 0..7
        gidx = np.zeros((128, NCHT * 8), np.int16)
        dloc = dl.reshape(NCHT, BLK).T  # [128, NCHT]
        for base, ncht in ((0, NCHTA), (NCHTA, NCHTB)):
            for s0 in range(0, ncht, MAXCH):
                sch = min(MAXCH, ncht - s0)
                L = sch * BLK
                seg = rows[(base + s0) * BLK : (base + s0) * BLK + L]
                wr = seg.reshape(L // 16, 16).T.astype(np.int16)  # [16, L//16]
                gidx[:, (base + s0) * 8 : (base + s0 + sch) * 8] = np.tile(wr, (8, 1))
        degp = np.concatenate(
            [deg[i * NP : (i + 1) * NP], np.ones(NBLK * BLK - NP, np.float32)]
        )
        per_core.append(
            {
                "x_tr": np.ascontiguousarray(x[i * NP : (i + 1) * NP].T),
                "deg_own": np.ascontiguousarray(degp.reshape(NBLK, BLK).T),
                "gidx": gidx,
                "dloc": np.ascontiguousarray(dloc).astype(ml_dtypes.bfloat16),
            }
        )

    meta = {
        "IN": IN,
        "CHA": [int(c) for c in CHA],
        "CHB": [int(c) for c in CHB],
        "cofsA": [int(c) for c in cofsA],
        "cofsB": [int(c) for c in cofsB],
        "NCHTA": NCHTA,
        "NCHTB": NCHTB,
        "NCHT": NCHT,
        "widths": widths,
    }
    return per_core, meta


def build_nc(meta, HID, OUT, ncores=NCORES):
    IN = meta["IN"]
    widths = meta["widths"]
    NCHT = meta["NCHT"]
    KC = IN // 128
    assert IN % 128 == 0 and HID == 128 and OUT <= 512

    nc = bacc.Bacc(
        "TRN2",
        target_bir_lowering=False,
        debug=False,
        num_devices=ncores,
        num_swdge_queues=NQUEUES,
    )

    x_tr = nc.dram_tensor("x_tr", [IN, NP], F32, kind="ExternalInput")
    w1 = nc.dram_tensor("w1", [IN, HID], F32, kind="ExternalInput")
    b1 = nc.dram_tensor("b1", [1, HID], F32, kind="ExternalInput")
    w2 = nc.dram_tensor("w2", [HID, OUT], F32, kind="ExternalInput")
    b2 = nc.dram_tensor("b2", [1, OUT], F32, kind="ExternalInput")
    deg_own = nc.dram_tensor("deg_own", [128, NBLK], F32, kind="ExternalInput")
    gidx_d = nc.dram_tensor("gidx", [128, NCHT * 8], I16, kind="ExternalInput")
    dloc_d = nc.dram_tensor("dloc", [128, NCHT], BF16, kind="ExternalInput")
    iota_d = nc.dram_tensor("iota", [128, MAXCH * 128], BF16, kind="ExternalInput")
    ident_d = nc.dram_tensor("ident", [128, 128], BF16, kind="ExternalInput")
    y = nc.dram_tensor("y", [NP, OUT], F32, kind="ExternalOutput")

    hs1_stage = nc.dram_tensor("hs1_stage", [NP, HID], BF16)
    hs1A = nc.dram_tensor("hs1A", [ncores * RA, HID], BF16, addr_space="Shared")
    hs1B = nc.dram_tensor("hs1B", [ncores * RB, HID], BF16, addr_space="Shared")
    hsr_stage = nc.dram_tensor("hsr_stage", [NP, HID], BF16)
    hsrA = nc.dram_tensor("hsrA", [ncores * RA, HID], BF16, addr_space="Shared")
    hsrB = nc.dram_tensor("hsrB", [ncores * RB, HID], BF16, addr_space="Shared")

    rg = [list(range(ncores))]
    qn = [0]

    def next_q():
        q = qn[0]
        qn[0] = (q + 1) % NQUEUES
        return q

    streams = [
        # (base chunk offset, cofs, ncht)
        (0, meta["cofsA"], meta["NCHTA"]),
        (meta["NCHTA"], meta["cofsB"], meta["NCHTB"]),
    ]

    with tile.TileContext(nc) as tc:
        nc.gpsimd.load_library(library_config.mlp)
        with (
            tc.tile_pool(name="const", bufs=1) as constp,
            tc.tile_pool(name="gath", bufs=8) as gathp,
            tc.tile_pool(name="oh", bufs=16) as ohp,
            tc.tile_pool(name="hs", bufs=8) as hsp,
            tc.tile_pool(name="sa", bufs=1) as sap,
            tc.tile_pool(name="ps", bufs=4, space="PSUM") as psp,
            tc.tile_pool(name="pso", bufs=2, space="PSUM") as psop,
        ):
            # ---- constants (DMA program order = priority order) ----
            w1c = []
            for k in range(KC):
                t = constp.tile([128, HID], F32, tag=f"w1c{k}")
                nc.sync.dma_start(out=t[:], in_=w1[k * 128 : (k + 1) * 128, :])
                w1c.append(t)
            # x in column chunks so phase B can start before the full load
            nxt_chunks = _cdiv(NP, XCHUNK)
            xsb = [[None] * nxt_chunks for _ in range(KC)]
            for t_i in range(nxt_chunks):
                c0 = t_i * XCHUNK
                cw = min(XCHUNK, NP - c0)
                for k in range(KC):
                    t = constp.tile([128, cw], F32, tag=f"x{k}_{t_i}")
                    nc.sync.dma_start(
                        out=t[:], in_=x_tr[k * 128 : (k + 1) * 128, c0 : c0 + cw]
                    )
                    xsb[k][t_i] = t
            b1_sb = constp.tile([1, HID], F32, tag="b1")
            nc.sync.dma_start(out=b1_sb[:], in_=b1[:, :])
            dinv_sb = constp.tile([128, NBLK], F32, tag="dinv")
            nc.sync.dma_start(out=dinv_sb[:], in_=deg_own[:, :])
            nc.scalar.sqrt(dinv_sb[:], dinv_sb[:])
            nc.vector.reciprocal(dinv_sb[:], dinv_sb[:])
            gidx_sb = constp.tile([128, NCHT * 8], I16, tag="gidx")
            nc.sync.dma_start(out=gidx_sb[:], in_=gidx_d[:, :])
            dloc_sb = constp.tile([128, NCHT], BF16, tag="dloc")
            nc.sync.dma_start(out=dloc_sb[:], in_=dloc_d[:, :])
            w2_sb = constp.tile([HID, OUT], F32, tag="w2")
            nc.sync.dma_start(out=w2_sb[:], in_=w2[:, :])
            b2_sb = constp.tile([1, OUT], F32, tag="b2")
            nc.sync.dma_start(out=b2_sb[:], in_=b2[:, :])

            ones_sb = constp.tile([1, 128], F32, tag="ones")
            nc.vector.memset(ones_sb[:], 1.0)
            pb = psop.tile([128, HID], F32, tag="po")
            nc.tensor.matmul(pb[:], lhsT=ones_sb[:], rhs=b1_sb[:],
                             start=True, stop=True)
            b1_bc = constp.tile([128, HID], F32, tag="b1bc")
            nc.vector.tensor_copy(b1_bc[:], pb[:])
            pb2 = psop.tile([128, OUT], F32, tag="po")
            nc.tensor.matmul(pb2[:], lhsT=ones_sb[:], rhs=b2_sb[:],
                             start=True, stop=True)
            b2_bc = constp.tile([128, OUT], F32, tag="b2bc")
            nc.vector.tensor_copy(b2_bc[:], pb2[:])

            iota_t = constp.tile([128, MAXCH * 128], BF16, tag="iott")
            nc.sync.dma_start(out=iota_t[:], in_=iota_d[:, :])
            ident_sb = constp.tile([128, 128], BF16, tag="ident")
            nc.sync.dma_start(out=ident_sb[:], in_=ident_d[:, :])

            # ---- phase B: hs1 = dinv * (x @ W1) for own rows ----
            for b in range(NBLK):
                w = widths[b]
                t_i = (b * BLK) // XCHUNK
                co = b * BLK - t_i * XCHUNK
                ph = psp.tile([128, HID], F32, tag="acc")
                for k in range(KC):
                    nc.tensor.matmul(
                        ph[:w, :],
                        lhsT=xsb[k][t_i][:, co : co + w],
                        rhs=w1c[k][:, :],
                        start=(k == 0),
                        stop=(k == KC - 1),
                    )
                hs1_t = hsp.tile([128, HID], BF16, tag="hs1")
                nc.scalar.activation(
                    hs1_t[:w, :],
                    ph[:w, :],
                    mybir.ActivationFunctionType.Copy,
                    scale=dinv_sb[:w, b : b + 1],
                )
                nc.sync.dma_start(
                    out=hs1_stage[b * BLK : b * BLK + w, :], in_=hs1_t[:w, :]
                )
                if b == HALF_BLKS - 1:
                    nc.gpsimd.collective_compute(
                        "AllGather",
                        mybir.AluOpType.bypass,
                        replica_groups=rg,
                        ins=[hs1_stage[0:RA, :].opt()],
                        outs=[hs1A[:, :].opt()],
                    )
            nc.gpsimd.collective_compute(
                "AllGather",
                mybir.AluOpType.bypass,
                replica_groups=rg,
                ins=[hs1_stage[RA:NP, :].opt()],
                outs=[hs1B[:, :].opt()],
            )

            def do_pass(table, stream_i, feat_major, close_block, add_sa=None):
                """One scatter pass over a gather stream.

                feat_major=False: psum[d, f] += oh^T @ g   (layer-1 layout)
                feat_major=True : psum[f, d] += g^T @ oh   (layer-2 layout)
                add_sa: per-block bf16 partials folded into the open PSUM
                group via an identity matmul before it closes.
                close_block(b, psum_tile) consumes the finished accumulator.
                """
                base, cofs, ncht = streams[stream_i]
                p = None
                for s0 in range(0, ncht, MAXCH):
                    sch = min(MAXCH, ncht - s0)
                    g = gathp.tile([128, MAXCH, HID], BF16, tag="g")
                    nc.gpsimd.dma_gather(
                        g[:, :sch, :],
                        table.ap(),
                        gidx_sb[:, (base + s0) * 8 : (base + s0 + sch) * 8],
                        sch * 128,
                        sch * 128,
                        HID,
                        queue_num=next_q(),
                    )
                    oh_seg = ohp.tile([128, MAXCH * 128], BF16, tag="oh")
                    nc.vector.tensor_tensor(
                        out=oh_seg[:, : sch * 128],
                        in0=dloc_sb[:, base + s0 : base + s0 + sch].to_broadcast(
                            [128, sch, 128]
                        ),
                        in1=iota_t[:, : sch * 128],
                        op=mybir.AluOpType.is_equal,
                    )
                    for c in range(sch):
                        ac = s0 + c
                        b = int(np.searchsorted(cofs, ac, side="right")) - 1
                        w = widths[b]
                        first = ac == cofs[b]
                        last = ac == cofs[b + 1] - 1
                        if first:
                            p = psp.tile([128, 128], F32, tag="acc")
                        stop_here = last and add_sa is None
                        if feat_major:
                            nc.tensor.matmul(
                                p[:, :w],
                                lhsT=g[:, c, :],
                                rhs=oh_seg[:, c * 128 : c * 128 + w],
                                start=first,
                                stop=stop_here,
                            )
                        else:
                            nc.tensor.matmul(
                                p[:w, :],
                                lhsT=oh_seg[:, c * 128 : c * 128 + w],
                                rhs=g[:, c, :],
                                start=first,
                                stop=stop_here,
                            )
                        if last:
                            if add_sa is not None:
                                if feat_major:
                                    nc.tensor.matmul(
                                        p[:, :w],
                                        lhsT=ident_sb[:, :],
                                        rhs=add_sa[b][:, :w],
                                        start=False,
                                        stop=True,
                                    )
                                else:
                                    nc.tensor.matmul(
                                        p[:w, :],
                                        lhsT=ident_sb[:, :w],
                                        rhs=add_sa[b][:, :],
                                        start=False,
                                        stop=True,
                                    )
                            close_block(b, p)

            # ---- phase D: layer-1 aggregation -> hsr ----
            sa1 = [sap.tile([128, HID], BF16, tag=f"sa1_{b}", name=f"sa1_{b}") for b in range(NBLK)]

            def d_close_a(b, p):
                w = widths[b]
                nc.scalar.activation(
                    sa1[b][:w, :], p[:w, :], mybir.ActivationFunctionType.Copy
                )

            def d_close_b(b, p):
                w = widths[b]
                t1 = hsp.tile([128, HID], F32, tag="t1")
                nc.scalar.activation(
                    t1[:w, :], p[:w, :],
                    mybir.ActivationFunctionType.Copy,
                    scale=dinv_sb[:w, b : b + 1],
                )
                t2 = hsp.tile([128, HID], F32, tag="t2")
                nc.vector.tensor_tensor(
                    out=t2[:w, :], in0=t1[:w, :], in1=b1_bc[:w, :],
                    op=mybir.AluOpType.add,
                )
                hsr_t = hsp.tile([128, HID], BF16, tag="hsr")
                nc.scalar.activation(
                    hsr_t[:w, :], t2[:w, :],
                    mybir.ActivationFunctionType.Relu,
                    scale=dinv_sb[:w, b : b + 1],
                )
                nc.sync.dma_start(
                    out=hsr_stage[b * BLK : b * BLK + w, :], in_=hsr_t[:w, :]
                )
                if b == HALF_BLKS - 1:
                    nc.gpsimd.collective_compute(
                        "AllGather",
                        mybir.AluOpType.bypass,
                        replica_groups=rg,
                        ins=[hsr_stage[0:RA, :].opt()],
                        outs=[hsrA[:, :].opt()],
                    )
                if b == NBLK - 1:
                    nc.gpsimd.collective_compute(
                        "AllGather",
                        mybir.AluOpType.bypass,
                        replica_groups=rg,
                        ins=[hsr_stage[RA:NP, :].opt()],
                        outs=[hsrB[:, :].opt()],
                    )

            do_pass(hs1A, 0, False, d_close_a)
            do_pass(hs1B, 1, False, d_close_b, add_sa=sa1)

            # ---- phase F: layer-2 aggregation -> y ----
            sa2 = [sap.tile([128, 128], BF16, tag=f"sa2_{b}", name=f"sa2_{b}") for b in range(NBLK)]

            def f_close_a(b, p):
                w = widths[b]
                nc.scalar.activation(
                    sa2[b][:, :w], p[:, :w], mybir.ActivationFunctionType.Copy
                )

            def f_close_b(b, p):
                w = widths[b]
                aggT = hsp.tile([128, 128], F32, tag="aggT")
                nc.scalar.activation(
                    aggT[:, :w], p[:, :w], mybir.ActivationFunctionType.Copy
                )
                po = psop.tile([128, OUT], F32, tag="po")
                nc.tensor.matmul(
                    po[:w, :], lhsT=aggT[:, :w], rhs=w2_sb[:, :],
                    start=True, stop=True,
                )
                o1 = hsp.tile([128, OUT], F32, tag="o1")
                nc.scalar.activation(
                    o1[:w, :], po[:w, :],
                    mybir.ActivationFunctionType.Copy,
                    scale=dinv_sb[:w, b : b + 1],
                )
                yt = hsp.tile([128, OUT], F32, tag="yt")
                nc.vector.tensor_tensor(
                    out=yt[:w, :], in0=o1[:w, :], in1=b2_bc[:w, :],
                    op=mybir.AluOpType.add,
                )
                nc.sync.dma_start(out=y[b * BLK : b * BLK + w, :], in_=yt[:w, :])

            do_pass(hsrA, 0, True, f_close_a)
            do_pass(hsrB, 1, True, f_close_b, add_sa=sa2)

    nc.compile()
    return nc


def make_in_maps(per_core, W1, b1, W2, b2):
    import ml_dtypes

    W1 = np.ascontiguousarray(np.asarray(W1, np.float32))
    W2 = np.ascontiguousarray(np.asarray(W2, np.float32))
    b1 = np.asarray(b1, np.float32).reshape(1, -1)
    b2 = np.asarray(b2, np.float32).reshape(1, -1)
    iota = np.tile(np.arange(128, dtype=np.float32), MAXCH)
    iota = np.broadcast_to(iota, (128, MAXCH * 128)).astype(ml_dtypes.bfloat16)
    ident = np.eye(128, dtype=np.float32).astype(ml_dtypes.bfloat16)
    return [
        {
            "x_tr": pc["x_tr"],
            "w1": W1,
            "b1": b1,
            "w2": W2,
            "b2": b2,
            "deg_own": pc["deg_own"],
            "gidx": pc["gidx"],
            "dloc": pc["dloc"],
            "iota": np.ascontiguousarray(iota),
            "ident": np.ascontiguousarray(ident),
        }
        for pc in per_core
    ]


def kernel_run(x, edge_index, W1, b1, W2, b2, trace=False, tmpdir=None):
    x = np.ascontiguousarray(np.asarray(x, np.float32))
    per_core, meta = preprocess(x, edge_index)
    HID = np.asarray(W1).shape[1]
    OUT = np.asarray(W2).shape[1]
    nc = build_nc(meta, HID, OUT)
    in_maps = make_in_maps(per_core, W1, b1, W2, b2)
    res = run_bass_kernel_spmd(
        nc, in_maps, core_ids=list(range(NCORES)), trace=trace, tmpdir=tmpdir
    )
    out = np.concatenate([r["y"] for r in res.results], axis=0)
    return out, res


def kernel(x, edge_index, W1, b1, W2, b2):
    out, _ = kernel_run(x, edge_index, W1, b1, W2, b2)
    return out


# revision 10
# speedup vs baseline: 1.3721x; 1.0139x over previous
"""2-layer GCN encoder as a distributed Bass kernel on 8 TRN2 NeuronCores.

Decomposition (per core, nodes sharded by destination):
  hs1[v] = dinv[v] * (x[v] @ W1)                  (own rows -> split AllGather, bf16)
  S1[d]  = sum_{e: dst=d} hs1[src_e]              (SWDGE dma_gather + one-hot matmul)
  hsr    = dinv * relu(dinv*S1 + b1)              (own rows -> split AllGather, bf16)
  S2[d]  = sum_{e: dst=d} hsr[src_e]
  y[d]   = dinv[d]*(S2[d] @ W2) + b2              (W2 commutes with the sum)

vs the original version:
  - one-hot scatter matrices are generated ON DEVICE (iota + tensor_scalar
    is_equal with a per-partition dst-index scalar) instead of streaming
    ~39MB of precomputed one-hots from HBM per core.
  - self-loops are plain edges in the gather lists (no identity-matmul /
    staging-reload special case).
  - each AllGather is split in two halves (rows [0,1920) and [1920,3750) of
    every core) writing two separate shared tables; edges are partitioned by
    source half so half-A gathers only depend on the first collective and
    overlap the second.
  - x is loaded in column chunks so phase B starts ~4MB earlier.
  - per-core edge lists keep random source order (sorted-by-src makes all
    16 SDMA engines sweep the same HBM region in lockstep - bank conflicts
    halve gather bandwidth), and gather segments span dst-block boundaries
    (uniform 1024-index gathers).
"""

import numpy as np

import concourse.bass as bass
import concourse.bacc as bacc
import concourse.mybir as mybir
import concourse.tile as tile
from concourse import library_config
from concourse.bass_utils import run_bass_kernel_spmd

F32 = mybir.dt.float32
BF16 = mybir.dt.bfloat16
I16 = mybir.dt.int16

NCORES = 8
BLK = 128
N = 30000
NP = N // NCORES          # 3750
NBLK = (NP + BLK - 1) // BLK   # 30
HALF_BLKS = 15
RA = HALF_BLKS * BLK      # 1920 rows in half A
RB = NP - RA              # 1830 rows in half B
# Max 128-index chunks per dma_gather instruction: the SWDGE descriptor
# ring holds only ~100 descriptors per DMA engine and a gather generates
# num_idxs/16 per ring; >=1792 indices hangs the ring-reclaim wait. 1024 is
# known-safe.
MAXCH = 8
NQUEUES = 4
XCHUNK = 1024             # x load column-chunk (8 blocks)


def _cdiv(a, b):
    return (a + b - 1) // b


def preprocess(x, edge_index, ncores=NCORES):
    """Host-side graph partitioning: shard edges by dst core, split by source
    half, sort by source row, build wrapped SWDGE gather indices and the
    per-edge local-dst table used for on-device one-hot generation."""
    import ml_dtypes

    n, IN = x.shape
    assert n == N and N % ncores == 0

    src = np.asarray(edge_index[0], dtype=np.int64)
    dst = np.asarray(edge_index[1], dtype=np.int64)
    # self-loops as ordinary edges (PyG gcn_norm add_self_loops=True)
    loop = np.arange(N, dtype=np.int64)
    src = np.concatenate([src, loop])
    dst = np.concatenate([dst, loop])
    deg = np.bincount(dst, minlength=N).astype(np.float32)

    src_core = src // NP
    src_j = src % NP
    in_a = src_j < RA
    # table rows in the split-AllGather layout
    row_a = src_core * RA + src_j
    row_b = src_core * RB + (src_j - RA)

    per_core_lists = []
    cntA = np.zeros((ncores, NBLK), np.int64)
    cntB = np.zeros((ncores, NBLK), np.int64)
    for i in range(ncores):
        m = (dst >= i * NP) & (dst < (i + 1) * NP)
        es, ed = src[m], dst[m] - i * NP
        ia = in_a[m]
        ra, rb = row_a[m], row_b[m]
        blk = ed // BLK
        dl = ed % BLK
        lists = {}
        for b in range(NBLK):
            mb = blk == b
            for half, rows in (("A", ra), ("B", rb)):
                mh = mb & (ia if half == "A" else ~ia)
                r = rows[mh]
                d = dl[mh]
                lists[(b, half)] = (r, d)
                if half == "A":
                    cntA[i, b] = r.size
                else:
                    cntB[i, b] = r.size
        per_core_lists.append(lists)

    CHA = np.maximum(1, _cdiv(cntA.max(axis=0), BLK)).astype(np.int64)
    CHB = np.maximum(1, _cdiv(cntB.max(axis=0), BLK)).astype(np.int64)
    cofsA = np.concatenate([[0], np.cumsum(CHA)]).astype(np.int64)
    cofsB = np.concatenate([[0], np.cumsum(CHB)]).astype(np.int64)
    NCHTA = int(CHA.sum())
    NCHTB = int(CHB.sum())
    NCHT = NCHTA + NCHTB
    widths = [min(BLK, NP - b * BLK) for b in range(NBLK)]

    def build_stream(lists, half, CH, ncht):
        rows = np.zeros(ncht * BLK, np.int64)
        dl = np.full(ncht * BLK, -1.0, np.float32)
        c0 = 0
        for b in range(NBLK):
            r, d = lists[(b, half)]
            L = int(CH[b]) * BLK
            rows[c0 * BLK : c0 * BLK + r.size] = r
            dl[c0 * BLK : c0 * BLK + d.size] = d
            c0 += int(CH[b])
        return rows, dl

    per_core = []
    for i in range(ncores):
        ra_s, dla = build_stream(per_core_lists[i], "A", CHA, NCHTA)
        rb_s, dlb = build_stream(per_core_lists[i], "B", CHB, NCHTB)
        rows = np.concatenate([ra_s, rb_s])
        dl = np.concatenate([dla, dlb])
        assert rows.max() < 32768
        # wrapped SWDGE index layout, per gather segment: idx k of a segment
        # sits at [16*rep + k%16, seg_col0 + k//16] for rep in 0..7
        gidx = np.zeros((128, NCHT * 8), np.int16)
        dloc = dl.reshape(NCHT, BLK).T  # [128, NCHT]
        for base, ncht in ((0, NCHTA), (NCHTA, NCHTB)):
            for s0 in range(0, ncht, MAXCH):
                sch = min(MAXCH, ncht - s0)
                L = sch * BLK
                seg = rows[(base + s0) * BLK : (base + s0) * BLK + L]
                wr = seg.reshape(L // 16, 16).T.astype(np.int16)  # [16, L//16]
                gidx[:, (base + s0) * 8 : (base + s0 + sch) * 8] = np.tile(wr, (8, 1))
        degp = np.concatenate(
            [deg[i * NP : (i + 1) * NP], np.ones(NBLK * BLK - NP, np.float32)]
        )
        per_core.append(
            {
                "x_tr": np.ascontiguousarray(x[i * NP : (i + 1) * NP].T),
                "deg_own": np.ascontiguousarray(degp.reshape(NBLK, BLK).T),
                "gidx": gidx,
                "dloc": np.ascontiguousarray(dloc).astype(ml_dtypes.bfloat16),
            }
        )

    meta = {
        "IN": IN,
        "CHA": [int(c) for c in CHA],
        "CHB": [int(c) for c in CHB],
        "cofsA": [int(c) for c in cofsA],
        "cofsB": [int(c) for c in cofsB],
        "NCHTA": NCHTA,
        "NCHTB": NCHTB,
        "NCHT": NCHT,
        "widths": widths,
    }
    return per_core, meta


def build_nc(meta, HID, OUT, ncores=NCORES):
    IN = meta["IN"]
    widths = meta["widths"]
    NCHT = meta["NCHT"]
    KC = IN // 128
    assert IN % 128 == 0 and HID == 128 and OUT <= 512

    nc = bacc.Bacc(
        "TRN2",
        target_bir_lowering=False,
        debug=False,
        num_devices=ncores,
        num_swdge_queues=NQUEUES,
    )

    x_tr = nc.dram_tensor("x_tr", [IN, NP], F32, kind="ExternalInput")
    w1 = nc.dram_tensor("w1", [IN, HID], F32, kind="ExternalInput")
    b1 = nc.dram_tensor("b1", [1, HID], F32, kind="ExternalInput")
    w2 = nc.dram_tensor("w2", [HID, OUT], F32, kind="ExternalInput")
    b2 = nc.dram_tensor("b2", [1, OUT], F32, kind="ExternalInput")
    deg_own = nc.dram_tensor("deg_own", [128, NBLK], F32, kind="ExternalInput")
    gidx_d = nc.dram_tensor("gidx", [128, NCHT * 8], I16, kind="ExternalInput")
    dloc_d = nc.dram_tensor("dloc", [128, NCHT], BF16, kind="ExternalInput")
    iota_d = nc.dram_tensor("iota", [128, MAXCH * 128], BF16, kind="ExternalInput")
    ident_d = nc.dram_tensor("ident", [128, 128], F32, kind="ExternalInput")
    y = nc.dram_tensor("y", [NP, OUT], F32, kind="ExternalOutput")

    hs1_stage = nc.dram_tensor("hs1_stage", [NP, HID], BF16)
    hs1A = nc.dram_tensor("hs1A", [ncores * RA, HID], BF16, addr_space="Shared")
    hs1B = nc.dram_tensor("hs1B", [ncores * RB, HID], BF16, addr_space="Shared")
    hsr_stage = nc.dram_tensor("hsr_stage", [NP, HID], BF16)
    hsrA = nc.dram_tensor("hsrA", [ncores * RA, HID], BF16, addr_space="Shared")
    hsrB = nc.dram_tensor("hsrB", [ncores * RB, HID], BF16, addr_space="Shared")

    rg = [list(range(ncores))]
    qn = [0]

    def next_q():
        q = qn[0]
        qn[0] = (q + 1) % NQUEUES
        return q

    streams = [
        # (base chunk offset, cofs, ncht)
        (0, meta["cofsA"], meta["NCHTA"]),
        (meta["NCHTA"], meta["cofsB"], meta["NCHTB"]),
    ]

    with tile.TileContext(nc) as tc:
        nc.gpsimd.load_library(library_config.mlp)
        with (
            tc.tile_pool(name="const", bufs=1) as constp,
            tc.tile_pool(name="gath", bufs=8) as gathp,
            tc.tile_pool(name="oh", bufs=16) as ohp,
            tc.tile_pool(name="hs", bufs=8) as hsp,
            tc.tile_pool(name="sa", bufs=1) as sap,
            tc.tile_pool(name="ps", bufs=4, space="PSUM") as psp,
            tc.tile_pool(name="pso", bufs=2, space="PSUM") as psop,
        ):
            # ---- constants (DMA program order = priority order) ----
            w1c = []
            for k in range(KC):
                t = constp.tile([128, HID], F32, tag=f"w1c{k}")
                nc.sync.dma_start(out=t[:], in_=w1[k * 128 : (k + 1) * 128, :])
                w1c.append(t)
            # x in column chunks so phase B can start before the full load
            nxt_chunks = _cdiv(NP, XCHUNK)
            xsb = [[None] * nxt_chunks for _ in range(KC)]
            for t_i in range(nxt_chunks):
                c0 = t_i * XCHUNK
                cw = min(XCHUNK, NP - c0)
                for k in range(KC):
                    t = constp.tile([128, cw], F32, tag=f"x{k}_{t_i}")
                    nc.sync.dma_start(
                        out=t[:], in_=x_tr[k * 128 : (k + 1) * 128, c0 : c0 + cw]
                    )
                    xsb[k][t_i] = t
            b1_sb = constp.tile([1, HID], F32, tag="b1")
            nc.sync.dma_start(out=b1_sb[:], in_=b1[:, :])
            dinv_sb = constp.tile([128, NBLK], F32, tag="dinv")
            nc.sync.dma_start(out=dinv_sb[:], in_=deg_own[:, :])
            nc.scalar.sqrt(dinv_sb[:], dinv_sb[:])
            nc.vector.reciprocal(dinv_sb[:], dinv_sb[:])
            gidx_sb = constp.tile([128, NCHT * 8], I16, tag="gidx")
            nc.sync.dma_start(out=gidx_sb[:], in_=gidx_d[:, :])
            dloc_sb = constp.tile([128, NCHT], BF16, tag="dloc")
            nc.sync.dma_start(out=dloc_sb[:], in_=dloc_d[:, :])
            w2_sb = constp.tile([HID, OUT], F32, tag="w2")
            nc.sync.dma_start(out=w2_sb[:], in_=w2[:, :])
            b2_sb = constp.tile([1, OUT], F32, tag="b2")
            nc.sync.dma_start(out=b2_sb[:], in_=b2[:, :])

            ones_sb = constp.tile([1, 128], F32, tag="ones")
            nc.vector.memset(ones_sb[:], 1.0)
            pb = psop.tile([128, HID], F32, tag="po")
            nc.tensor.matmul(pb[:], lhsT=ones_sb[:], rhs=b1_sb[:],
                             start=True, stop=True)
            b1_bc = constp.tile([128, HID], F32, tag="b1bc")
            nc.vector.tensor_copy(b1_bc[:], pb[:])
            pb2 = psop.tile([128, OUT], F32, tag="po")
            nc.tensor.matmul(pb2[:], lhsT=ones_sb[:], rhs=b2_sb[:],
                             start=True, stop=True)
            b2_bc = constp.tile([128, OUT], F32, tag="b2bc")
            nc.vector.tensor_copy(b2_bc[:], pb2[:])

            iota_t = constp.tile([128, MAXCH * 128], BF16, tag="iott")
            nc.sync.dma_start(out=iota_t[:], in_=iota_d[:, :])
            ident_sb = constp.tile([128, 128], F32, tag="ident")
            nc.sync.dma_start(out=ident_sb[:], in_=ident_d[:, :])

            # ---- phase B: hs1 = dinv * (x @ W1) for own rows ----
            for b in range(NBLK):
                w = widths[b]
                t_i = (b * BLK) // XCHUNK
                co = b * BLK - t_i * XCHUNK
                ph = psp.tile([128, HID], F32, tag="acc")
                for k in range(KC):
                    nc.tensor.matmul(
                        ph[:w, :],
                        lhsT=xsb[k][t_i][:, co : co + w],
                        rhs=w1c[k][:, :],
                        start=(k == 0),
                        stop=(k == KC - 1),
                    )
                hs1_t = hsp.tile([128, HID], BF16, tag="hs1")
                nc.scalar.activation(
                    hs1_t[:w, :],
                    ph[:w, :],
                    mybir.ActivationFunctionType.Copy,
                    scale=dinv_sb[:w, b : b + 1],
                )
                nc.sync.dma_start(
                    out=hs1_stage[b * BLK : b * BLK + w, :], in_=hs1_t[:w, :]
                )
                if b == HALF_BLKS - 1:
                    nc.gpsimd.collective_compute(
                        "AllGather",
                        mybir.AluOpType.bypass,
                        replica_groups=rg,
                        ins=[hs1_stage[0:RA, :].opt()],
                        outs=[hs1A[:, :].opt()],
                    )
            nc.gpsimd.collective_compute(
                "AllGather",
                mybir.AluOpType.bypass,
                replica_groups=rg,
                ins=[hs1_stage[RA:NP, :].opt()],
                outs=[hs1B[:, :].opt()],
            )

            def do_pass(table, stream_i, feat_major, close_block, add_sa=None):
                """One scatter pass over a gather stream.

                feat_major=False: psum[d, f] += oh^T @ g   (layer-1 layout)
                feat_major=True : psum[f, d] += g^T @ oh   (layer-2 layout)
                add_sa: per-block bf16 partials folded into the open PSUM
                group via an identity matmul before it closes.
                close_block(b, psum_tile) consumes the finished accumulator.
                """
                base, cofs, ncht = streams[stream_i]
                p = None
                for s0 in range(0, ncht, MAXCH):
                    sch = min(MAXCH, ncht - s0)
                    g = gathp.tile([128, MAXCH, HID], BF16, tag="g")
                    nc.gpsimd.dma_gather(
                        g[:, :sch, :],
                        table.ap(),
                        gidx_sb[:, (base + s0) * 8 : (base + s0 + sch) * 8],
                        sch * 128,
                        sch * 128,
                        HID,
                        queue_num=next_q(),
                    )
                    oh_seg = ohp.tile([128, MAXCH * 128], BF16, tag="oh")
                    nc.vector.tensor_tensor(
                        out=oh_seg[:, : sch * 128],
                        in0=dloc_sb[:, base + s0 : base + s0 + sch].to_broadcast(
                            [128, sch, 128]
                        ),
                        in1=iota_t[:, : sch * 128],
                        op=mybir.AluOpType.is_equal,
                    )
                    for c in range(sch):
                        ac = s0 + c
                        b = int(np.searchsorted(cofs, ac, side="right")) - 1
                        w = widths[b]
                        first = ac == cofs[b]
                        last = ac == cofs[b + 1] - 1
                        if first:
                            p = psp.tile([128, 128], F32, tag="acc")
                        stop_here = last and add_sa is None
                        if feat_major:
                            nc.tensor.matmul(
                                p[:, :w],
                                lhsT=g[:, c, :],
                                rhs=oh_seg[:, c * 128 : c * 128 + w],
                                start=first,
                                stop=stop_here,
                            )
                        else:
                            nc.tensor.matmul(
                                p[:w, :],
                                lhsT=oh_seg[:, c * 128 : c * 128 + w],
                                rhs=g[:, c, :],
                                start=first,
                                stop=stop_here,
                            )
                        if last:
                            if add_sa is not None:
                                if feat_major:
                                    nc.tensor.matmul(
                                        p[:, :w],
                                        lhsT=ident_sb[:, :],
                                        rhs=add_sa[b][:, :w],
                                        start=False,
                                        stop=True,
                                    )
                                else:
                                    nc.tensor.matmul(
                                        p[:w, :],
                                        lhsT=ident_sb[:, :w],
                                        rhs=add_sa[b][:, :],
                                        start=False,
                                        stop=True,
                                    )
                            close_block(b, p)

            # ---- phase D: layer-1 aggregation -> hsr ----
            sa1 = [sap.tile([128, HID], F32, tag=f"sa1_{b}", name=f"sa1_{b}") for b in range(NBLK)]

            def d_close_a(b, p):
                w = widths[b]
                nc.scalar.activation(
                    sa1[b][:w, :], p[:w, :], mybir.ActivationFunctionType.Copy
                )

            def d_close_b(b, p):
                w = widths[b]
                t1 = hsp.tile([128, HID], F32, tag="t1")
                nc.scalar.activation(
                    t1[:w, :], p[:w, :],
                    mybir.ActivationFunctionType.Copy,
                    scale=dinv_sb[:w, b : b + 1],
                )
                t2 = hsp.tile([128, HID], F32, tag="t2")
                nc.vector.tensor_tensor(
                    out=t2[:w, :], in0=t1[:w, :], in1=b1_bc[:w, :],
                    op=mybir.AluOpType.add,
                )
                hsr_t = hsp.tile([128, HID], BF16, tag="hsr")
                nc.scalar.activation(
                    hsr_t[:w, :], t2[:w, :],
                    mybir.ActivationFunctionType.Relu,
                    scale=dinv_sb[:w, b : b + 1],
                )
                nc.sync.dma_start(
                    out=hsr_stage[b * BLK : b * BLK + w, :], in_=hsr_t[:w, :]
                )
                if b == HALF_BLKS - 1:
                    nc.gpsimd.collective_compute(
                        "AllGather",
                        mybir.AluOpType.bypass,
                        replica_groups=rg,
                        ins=[hsr_stage[0:RA, :].opt()],
                        outs=[hsrA[:, :].opt()],
                    )
                if b == NBLK - 1:
                    nc.gpsimd.collective_compute(
                        "AllGather",
                        mybir.AluOpType.bypass,
                        replica_groups=rg,
                        ins=[hsr_stage[RA:NP, :].opt()],
                        outs=[hsrB[:, :].opt()],
                    )

            do_pass(hs1A, 0, False, d_close_a)
            do_pass(hs1B, 1, False, d_close_b, add_sa=sa1)

            # ---- phase F: layer-2 aggregation -> y ----
            sa2 = [sap.tile([128, 128], F32, tag=f"sa2_{b}", name=f"sa2_{b}") for b in range(NBLK)]

            def f_close_a(b, p):
                w = widths[b]
                nc.scalar.activation(
                    sa2[b][:, :w], p[:, :w], mybir.ActivationFunctionType.Copy
                )

            def f_close_b(b, p):
                w = widths[b]
                aggT = hsp.tile([128, 128], F32, tag="aggT")
                nc.scalar.activation(
                    aggT[:, :w], p[:, :w], mybir.ActivationFunctionType.Copy
                )
                po = psop.tile([128, OUT], F32, tag="po")
                nc.tensor.matmul(
                    po[:w, :], lhsT=aggT[:, :w], rhs=w2_sb[:, :],
                    start=True, stop=True,
                )
                o1 = hsp.tile([128, OUT], F32, tag="o1")
                nc.scalar.activation(
                    o1[:w, :], po[:w, :],
                    mybir.ActivationFunctionType.Copy,
                    scale=dinv_sb[:w, b : b + 1],
                )
                yt = hsp.tile([128, OUT], F32, tag="yt")
                nc.vector.tensor_tensor(
                    out=yt[:w, :], in0=o1[:w, :], in1=b2_bc[:w, :],
                    op=mybir.AluOpType.add,
                )
                nc.sync.dma_start(out=y[b * BLK : b * BLK + w, :], in_=yt[:w, :])

            do_pass(hsrA, 0, True, f_close_a)
            do_pass(hsrB, 1, True, f_close_b, add_sa=sa2)

    nc.compile()
    return nc


def make_in_maps(per_core, W1, b1, W2, b2):
    import ml_dtypes

    W1 = np.ascontiguousarray(np.asarray(W1, np.float32))
    W2 = np.ascontiguousarray(np.asarray(W2, np.float32))
    b1 = np.asarray(b1, np.float32).reshape(1, -1)
    b2 = np.asarray(b2, np.float32).reshape(1, -1)
    iota = np.tile(np.arange(128, dtype=np.float32), MAXCH)
    iota = np.broadcast_to(iota, (128, MAXCH * 128)).astype(ml_dtypes.bfloat16)
    ident = np.eye(128, dtype=np.float32)
    return [
        {
            "x_tr": pc["x_tr"],
            "w1": W1,
            "b1": b1,
            "w2": W2,
            "b2": b2,
            "deg_own": pc["deg_own"],
            "gidx": pc["gidx"],
            "dloc": pc["dloc"],
            "iota": np.ascontiguousarray(iota),
            "ident": np.ascontiguousarray(ident),
        }
        for pc in per_core
    ]


def kernel_run(x, edge_index, W1, b1, W2, b2, trace=False, tmpdir=None):
    x = np.ascontiguousarray(np.asarray(x, np.float32))
    per_core, meta = preprocess(x, edge_index)
    HID = np.asarray(W1).shape[1]
    OUT = np.asarray(W2).shape[1]
    nc = build_nc(meta, HID, OUT)
    in_maps = make_in_maps(per_core, W1, b1, W2, b2)
    res = run_bass_kernel_spmd(
        nc, in_maps, core_ids=list(range(NCORES)), trace=trace, tmpdir=tmpdir
    )
    out = np.concatenate([r["y"] for r in res.results], axis=0)
    return out, res


def kernel(x, edge_index, W1, b1, W2, b2):
    out, _ = kernel_run(x, edge_index, W1, b1, W2, b2)
    return out


# revision 12
# speedup vs baseline: 1.6917x; 1.2329x over previous
"""2-layer GCN encoder as a distributed Bass kernel on 8 TRN2 NeuronCores.

Decomposition (per core, nodes sharded by destination):
  hs1[v] = dinv[v] * (x[v] @ W1)                  (own rows -> AllGather, bf16)
  S1[d]  = sum_{e: dst=d} hs1[src_e]              (SWDGE dma_gather + one-hot matmul)
  hsr    = dinv * relu(dinv*S1 + b1)              (own rows -> AllGather, bf16)
  S2[d]  = sum_{e: dst=d} hsr[src_e]
  y[d]   = dinv[d]*(S2[d] @ W2) + b2              (W2 commutes with the sum)

vs the original version:
  - one-hot scatter matrices are generated ON DEVICE, one tensor_tensor
    is_equal per 1024-edge gather segment (broadcast per-edge dst index vs a
    tiled iota), instead of streaming ~39MB of precomputed one-hots per core.
  - self-loops use an identity matmul against SBUF-resident own-block tiles
    (no gather traffic, no staging reload from DRAM).
  - x is loaded in column chunks so phase B starts earlier.
  - per-core edge lists keep random source order (sorted-by-src makes all
    16 SDMA engines sweep the same HBM region in lockstep - bank conflicts
    hurt gather bandwidth), and gather segments span dst-block boundaries
    (uniform 1024-index gathers).
"""

import numpy as np

import concourse.bass as bass
import concourse.bacc as bacc
import concourse.mybir as mybir
import concourse.tile as tile
from concourse import library_config
from concourse.bass_utils import run_bass_kernel_spmd

F32 = mybir.dt.float32
BF16 = mybir.dt.bfloat16
I16 = mybir.dt.int16

NCORES = 8
BLK = 128
N = 30000
NP = N // NCORES          # 3750
NBLK = (NP + BLK - 1) // BLK   # 30
# Max 128-index chunks per dma_gather instruction: the SWDGE descriptor
# ring holds only ~100 descriptors per DMA engine and a gather generates
# num_idxs/16 per ring; >=1792 indices hangs the ring-reclaim wait. 1024 is
# known-safe.
MAXCH = 8
NQUEUES = 4
XCHUNK = 1024             # x load column-chunk (8 blocks)


def _cdiv(a, b):
    return (a + b - 1) // b


def preprocess(x, edge_index, ncores=NCORES):
    """Host-side graph partitioning: shard edges by dst core, build wrapped
    SWDGE gather indices and the per-edge local-dst table used for on-device
    one-hot generation. Self-loops are handled on device, not gathered."""
    import ml_dtypes

    n, IN = x.shape
    assert n == N and N % ncores == 0

    src = np.asarray(edge_index[0], dtype=np.int64)
    dst = np.asarray(edge_index[1], dtype=np.int64)
    # degree includes the self-loop (PyG gcn_norm add_self_loops=True); the
    # self-loop itself is applied on device via an identity matmul
    deg = (np.bincount(dst, minlength=N) + 1).astype(np.float32)

    per_core_lists = []
    cnt = np.zeros((ncores, NBLK), np.int64)
    for i in range(ncores):
        m = (dst >= i * NP) & (dst < (i + 1) * NP)
        es, ed = src[m], dst[m] - i * NP
        blk = ed // BLK
        dl = ed % BLK
        lists = {}
        for b in range(NBLK):
            mb = blk == b
            lists[b] = (es[mb], dl[mb])
            cnt[i, b] = int(mb.sum())
        per_core_lists.append(lists)

    CH = np.maximum(1, _cdiv(cnt.max(axis=0), BLK)).astype(np.int64)
    cofs = np.concatenate([[0], np.cumsum(CH)]).astype(np.int64)
    NCHT = int(CH.sum())
    widths = [min(BLK, NP - b * BLK) for b in range(NBLK)]

    per_core = []
    for i in range(ncores):
        rows = np.zeros(NCHT * BLK, np.int64)
        dl_s = np.full(NCHT * BLK, -1.0, np.float32)
        c0 = 0
        for b in range(NBLK):
            r, d = per_core_lists[i][b]
            rows[c0 * BLK : c0 * BLK + r.size] = r
            dl_s[c0 * BLK : c0 * BLK + d.size] = d
            c0 += int(CH[b])
        assert rows.max() < 32768
        # wrapped SWDGE index layout, per gather segment: idx k of a segment
        # sits at [16*rep + k%16, seg_col0 + k//16] for rep in 0..7
        gidx = np.zeros((128, NCHT * 8), np.int16)
        dloc = dl_s.reshape(NCHT, BLK).T  # [128, NCHT]
        for s0 in range(0, NCHT, MAXCH):
            sch = min(MAXCH, NCHT - s0)
            L = sch * BLK
            seg = rows[s0 * BLK : s0 * BLK + L]
            wr = seg.reshape(L // 16, 16).T.astype(np.int16)  # [16, L//16]
            gidx[:, s0 * 8 : (s0 + sch) * 8] = np.tile(wr, (8, 1))
        degp = np.concatenate(
            [deg[i * NP : (i + 1) * NP], np.ones(NBLK * BLK - NP, np.float32)]
        )
        per_core.append(
            {
                "x_tr": np.ascontiguousarray(x[i * NP : (i + 1) * NP].T),
                "deg_own": np.ascontiguousarray(degp.reshape(NBLK, BLK).T),
                "gidx": gidx,
                "dloc": np.ascontiguousarray(dloc).astype(ml_dtypes.bfloat16),
            }
        )

    meta = {
        "IN": IN,
        "CH": [int(c) for c in CH],
        "cofs": [int(c) for c in cofs],
        "NCHT": NCHT,
        "widths": widths,
    }
    return per_core, meta


def build_nc(meta, HID, OUT, ncores=NCORES):
    IN = meta["IN"]
    widths = meta["widths"]
    NCHT = meta["NCHT"]
    cofs = meta["cofs"]
    KC = IN // 128
    assert IN % 128 == 0 and HID == 128 and OUT <= 512

    nc = bacc.Bacc(
        "TRN2",
        target_bir_lowering=False,
        debug=False,
        num_devices=ncores,
        num_swdge_queues=NQUEUES,
    )

    x_tr = nc.dram_tensor("x_tr", [IN, NP], F32, kind="ExternalInput")
    w1 = nc.dram_tensor("w1", [IN, HID], F32, kind="ExternalInput")
    b1 = nc.dram_tensor("b1", [1, HID], F32, kind="ExternalInput")
    w2 = nc.dram_tensor("w2", [HID, OUT], F32, kind="ExternalInput")
    b2 = nc.dram_tensor("b2", [1, OUT], F32, kind="ExternalInput")
    deg_own = nc.dram_tensor("deg_own", [128, NBLK], F32, kind="ExternalInput")
    gidx_d = nc.dram_tensor("gidx", [128, NCHT * 8], I16, kind="ExternalInput")
    dloc_d = nc.dram_tensor("dloc", [128, NCHT], BF16, kind="ExternalInput")
    iota_d = nc.dram_tensor("iota", [128, MAXCH * 128], BF16, kind="ExternalInput")
    ident_d = nc.dram_tensor("ident", [128, 128], BF16, kind="ExternalInput")
    y = nc.dram_tensor("y", [NP, OUT], F32, kind="ExternalOutput")

    hs1_stage = nc.dram_tensor("hs1_stage", [NP, HID], BF16)
    hs1_full = nc.dram_tensor("hs1_full", [N, HID], BF16, addr_space="Shared")
    hsr_stage = nc.dram_tensor("hsr_stage", [NP, HID], BF16)
    hsr_full = nc.dram_tensor("hsr_full", [N, HID], BF16, addr_space="Shared")

    rg = [list(range(ncores))]
    qn = [0]

    def next_q():
        q = qn[0]
        qn[0] = (q + 1) % NQUEUES
        return q

    with tile.TileContext(nc) as tc:
        nc.gpsimd.load_library(library_config.mlp)
        with (
            tc.tile_pool(name="const", bufs=1) as constp,
            tc.tile_pool(name="own", bufs=1) as ownp,
            tc.tile_pool(name="gath", bufs=8) as gathp,
            tc.tile_pool(name="oh", bufs=8) as ohp,
            tc.tile_pool(name="hs", bufs=8) as hsp,
            tc.tile_pool(name="ps", bufs=4, space="PSUM") as psp,
            tc.tile_pool(name="pso", bufs=2, space="PSUM") as psop,
        ):
            # ---- constants (DMA program order = priority order) ----
            w1c = []
            for k in range(KC):
                t = constp.tile([128, HID], F32, tag=f"w1c{k}", name=f"w1c{k}")
                nc.sync.dma_start(out=t[:], in_=w1[k * 128 : (k + 1) * 128, :])
                w1c.append(t)
            # x in column chunks so phase B can start before the full load
            nxt_chunks = _cdiv(NP, XCHUNK)
            xsb = [[None] * nxt_chunks for _ in range(KC)]
            for t_i in range(nxt_chunks):
                c0 = t_i * XCHUNK
                cw = min(XCHUNK, NP - c0)
                for k in range(KC):
                    t = constp.tile([128, cw], F32, tag=f"x{k}_{t_i}",
                                    name=f"x{k}_{t_i}")
                    nc.sync.dma_start(
                        out=t[:], in_=x_tr[k * 128 : (k + 1) * 128, c0 : c0 + cw]
                    )
                    xsb[k][t_i] = t
            b1_sb = constp.tile([1, HID], F32, tag="b1", name="b1_sb")
            nc.sync.dma_start(out=b1_sb[:], in_=b1[:, :])
            dinv_sb = constp.tile([128, NBLK], F32, tag="dinv", name="dinv_sb")
            nc.sync.dma_start(out=dinv_sb[:], in_=deg_own[:, :])
            nc.scalar.sqrt(dinv_sb[:], dinv_sb[:])
            nc.vector.reciprocal(dinv_sb[:], dinv_sb[:])
            gidx_sb = constp.tile([128, NCHT * 8], I16, tag="gidx", name="gidx_sb")
            nc.sync.dma_start(out=gidx_sb[:], in_=gidx_d[:, :])
            dloc_sb = constp.tile([128, NCHT], BF16, tag="dloc", name="dloc_sb")
            nc.sync.dma_start(out=dloc_sb[:], in_=dloc_d[:, :])
            iota_t = constp.tile([128, MAXCH * 128], BF16, tag="iott", name="iota_t")
            nc.sync.dma_start(out=iota_t[:], in_=iota_d[:, :])
            ident_sb = constp.tile([128, 128], BF16, tag="ident", name="ident_sb")
            nc.sync.dma_start(out=ident_sb[:], in_=ident_d[:, :])
            w2_sb = constp.tile([HID, OUT], F32, tag="w2", name="w2_sb")
            nc.sync.dma_start(out=w2_sb[:], in_=w2[:, :])
            b2_sb = constp.tile([1, OUT], F32, tag="b2", name="b2_sb")
            nc.sync.dma_start(out=b2_sb[:], in_=b2[:, :])

            ones_sb = constp.tile([1, 128], F32, tag="ones", name="ones_sb")
            nc.vector.memset(ones_sb[:], 1.0)
            pb = psop.tile([128, HID], F32, tag="po", name="pb")
            nc.tensor.matmul(pb[:], lhsT=ones_sb[:], rhs=b1_sb[:],
                             start=True, stop=True)
            b1_bc = constp.tile([128, HID], F32, tag="b1bc", name="b1_bc")
            nc.vector.tensor_copy(b1_bc[:], pb[:])
            pb2 = psop.tile([128, OUT], F32, tag="po", name="pb2")
            nc.tensor.matmul(pb2[:], lhsT=ones_sb[:], rhs=b2_sb[:],
                             start=True, stop=True)
            b2_bc = constp.tile([128, OUT], F32, tag="b2bc", name="b2_bc")
            nc.vector.tensor_copy(b2_bc[:], pb2[:])

            # ---- phase B: hs1 = dinv * (x @ W1); own blocks stay in SBUF ----
            hs1_own = [
                ownp.tile([128, HID], BF16, tag=f"hso_{b}", name=f"hso_{b}")
                for b in range(NBLK)
            ]
            hsr_own = [
                ownp.tile([128, HID], BF16, tag=f"hro_{b}", name=f"hro_{b}")
                for b in range(NBLK)
            ]
            for b in range(NBLK):
                w = widths[b]
                t_i = (b * BLK) // XCHUNK
                co = b * BLK - t_i * XCHUNK
                ph = psp.tile([128, HID], F32, tag="acc", name="ph")
                for k in range(KC):
                    nc.tensor.matmul(
                        ph[:w, :],
                        lhsT=xsb[k][t_i][:, co : co + w],
                        rhs=w1c[k][:, :],
                        start=(k == 0),
                        stop=(k == KC - 1),
                    )
                nc.scalar.activation(
                    hs1_own[b][:w, :],
                    ph[:w, :],
                    mybir.ActivationFunctionType.Copy,
                    scale=dinv_sb[:w, b : b + 1],
                )
                nc.sync.dma_start(
                    out=hs1_stage[b * BLK : b * BLK + w, :], in_=hs1_own[b][:w, :]
                )
            nc.gpsimd.collective_compute(
                "AllGather",
                mybir.AluOpType.bypass,
                replica_groups=rg,
                ins=[hs1_stage[:, :].opt()],
                outs=[hs1_full[:, :].opt()],
            )

            def do_pass(table, own, feat_major, close_block):
                """One scatter pass over the gather stream.

                feat_major=False: psum[d, f] += oh^T @ g   (layer-1 layout)
                feat_major=True : psum[f, d] += g^T @ oh   (layer-2 layout)
                The self-loop is an identity matmul against the SBUF-resident
                own-block tile, opening each block's PSUM group.
                """
                p = None
                for s0 in range(0, NCHT, MAXCH):
                    sch = min(MAXCH, NCHT - s0)
                    g = gathp.tile([128, MAXCH, HID], BF16, tag="g", name="g")
                    nc.gpsimd.dma_gather(
                        g[:, :sch, :],
                        table.ap(),
                        gidx_sb[:, s0 * 8 : (s0 + sch) * 8],
                        sch * 128,
                        sch * 128,
                        HID,
                        queue_num=next_q(),
                    )
                    oh_seg = ohp.tile([128, MAXCH * 128], BF16, tag="oh",
                                      name="oh_seg")
                    nc.vector.tensor_tensor(
                        out=oh_seg[:, : sch * 128],
                        in0=dloc_sb[:, s0 : s0 + sch].to_broadcast([128, sch, 128]),
                        in1=iota_t[:, : sch * 128],
                        op=mybir.AluOpType.is_equal,
                    )
                    for c in range(sch):
                        ac = s0 + c
                        b = int(np.searchsorted(cofs, ac, side="right")) - 1
                        w = widths[b]
                        first = ac == cofs[b]
                        last = ac == cofs[b + 1] - 1
                        if first:
                            p = psp.tile([128, 128], F32, tag="acc", name="p_acc")
                            # self-loop via identity matmul opens the group
                            if feat_major:
                                nc.tensor.matmul(
                                    p[:, :w],
                                    lhsT=own[b][:w, :],
                                    rhs=ident_sb[:w, :w],
                                    start=True,
                                    stop=False,
                                )
                            else:
                                nc.tensor.matmul(
                                    p[:w, :],
                                    lhsT=ident_sb[:w, :w],
                                    rhs=own[b][:w, :],
                                    start=True,
                                    stop=False,
                                )
                        if feat_major:
                            nc.tensor.matmul(
                                p[:, :w],
                                lhsT=g[:, c, :],
                                rhs=oh_seg[:, c * 128 : c * 128 + w],
                                start=False,
                                stop=last,
                            )
                        else:
                            nc.tensor.matmul(
                                p[:w, :],
                                lhsT=oh_seg[:, c * 128 : c * 128 + w],
                                rhs=g[:, c, :],
                                start=False,
                                stop=last,
                            )
                        if last:
                            close_block(b, p)

            # ---- phase D: layer-1 aggregation -> hsr ----
            def d_close(b, p):
                w = widths[b]
                t1 = hsp.tile([128, HID], F32, tag="t1", name="t1")
                nc.scalar.activation(
                    t1[:w, :], p[:w, :],
                    mybir.ActivationFunctionType.Copy,
                    scale=dinv_sb[:w, b : b + 1],
                )
                t2 = hsp.tile([128, HID], F32, tag="t2", name="t2")
                nc.vector.tensor_tensor(
                    out=t2[:w, :], in0=t1[:w, :], in1=b1_bc[:w, :],
                    op=mybir.AluOpType.add,
                )
                nc.scalar.activation(
                    hsr_own[b][:w, :], t2[:w, :],
                    mybir.ActivationFunctionType.Relu,
                    scale=dinv_sb[:w, b : b + 1],
                )
                nc.sync.dma_start(
                    out=hsr_stage[b * BLK : b * BLK + w, :], in_=hsr_own[b][:w, :]
                )
                if b == NBLK - 1:
                    nc.gpsimd.collective_compute(
                        "AllGather",
                        mybir.AluOpType.bypass,
                        replica_groups=rg,
                        ins=[hsr_stage[:, :].opt()],
                        outs=[hsr_full[:, :].opt()],
                    )

            do_pass(hs1_full, hs1_own, False, d_close)

            # ---- phase F: layer-2 aggregation -> y ----
            def f_close(b, p):
                w = widths[b]
                aggT = hsp.tile([128, 128], F32, tag="aggT", name="aggT")
                nc.scalar.activation(
                    aggT[:, :w], p[:, :w], mybir.ActivationFunctionType.Copy
                )
                po = psop.tile([128, OUT], F32, tag="po", name="po")
                nc.tensor.matmul(
                    po[:w, :], lhsT=aggT[:, :w], rhs=w2_sb[:, :],
                    start=True, stop=True,
                )
                o1 = hsp.tile([128, OUT], F32, tag="o1", name="o1")
                nc.scalar.activation(
                    o1[:w, :], po[:w, :],
                    mybir.ActivationFunctionType.Copy,
                    scale=dinv_sb[:w, b : b + 1],
                )
                yt = hsp.tile([128, OUT], F32, tag="yt", name="yt")
                nc.vector.tensor_tensor(
                    out=yt[:w, :], in0=o1[:w, :], in1=b2_bc[:w, :],
                    op=mybir.AluOpType.add,
                )
                nc.sync.dma_start(out=y[b * BLK : b * BLK + w, :], in_=yt[:w, :])

            do_pass(hsr_full, hsr_own, True, f_close)

    nc.compile()
    return nc


def make_in_maps(per_core, W1, b1, W2, b2):
    import ml_dtypes

    W1 = np.ascontiguousarray(np.asarray(W1, np.float32))
    W2 = np.ascontiguousarray(np.asarray(W2, np.float32))
    b1 = np.asarray(b1, np.float32).reshape(1, -1)
    b2 = np.asarray(b2, np.float32).reshape(1, -1)
    iota = np.tile(np.arange(128, dtype=np.float32), MAXCH)
    iota = np.broadcast_to(iota, (128, MAXCH * 128)).astype(ml_dtypes.bfloat16)
    ident = np.eye(128, dtype=np.float32).astype(ml_dtypes.bfloat16)
    return [
        {
            "x_tr": pc["x_tr"],
            "w1": W1,
            "b1": b1,
            "w2": W2,
            "b2": b2,
            "deg_own": pc["deg_own"],
            "gidx": pc["gidx"],
            "dloc": pc["dloc"],
            "iota": np.ascontiguousarray(iota),
            "ident": np.ascontiguousarray(ident),
        }
        for pc in per_core
    ]


def kernel_run(x, edge_index, W1, b1, W2, b2, trace=False, tmpdir=None):
    x = np.ascontiguousarray(np.asarray(x, np.float32))
    per_core, meta = preprocess(x, edge_index)
    HID = np.asarray(W1).shape[1]
    OUT = np.asarray(W2).shape[1]
    nc = build_nc(meta, HID, OUT)
    in_maps = make_in_maps(per_core, W1, b1, W2, b2)
    res = run_bass_kernel_spmd(
        nc, in_maps, core_ids=list(range(NCORES)), trace=trace, tmpdir=tmpdir
    )
    out = np.concatenate([r["y"] for r in res.results], axis=0)
    return out, res


def kernel(x, edge_index, W1, b1, W2, b2):
    out, _ = kernel_run(x, edge_index, W1, b1, W2, b2)
    return out
